# revision 1
# baseline (speedup 1.0000x reference)
"""BiLIF (bidirectional leaky-integrate-and-fire) node on 8 Trainium2 NeuronCores.

Problem: inputs [T=16, B=64, N=65536] f32.
  s1 = LIF-scan(x,          tau=4/3, v_th=0.75)   (hard reset to 0)
  s2 = LIF-scan(flip(x, 0), tau=4/3, v_th=1.25)
  out = (s1 + s2) / 2

Strategy
  - Shard the batch dim across the 8 cores (pure data parallel, no
    collectives). Per core: 8*65536 = 524288 positions = 128 partitions x
    4096 columns, processed as 2 column-chunks of [128, 2048].
  - The time recurrence is sequential and local. Both direction scans run
    concurrently: at step t the forward scan consumes x[t] and the backward
    scan consumes x[15-t], so out[t] completes at step t and every x tile is
    loaded exactly once (input read once, output written once).
  - The whole LIF step (reset -> charge) is ONE fused custom DVE op:
        h_new = (x - vp)*r + vp,   vp = select(h_prev < v_th, h_prev, 0)
    with r = fl32(1/tau) = 0.75 exactly, keeping device arithmetic within
    1 ulp of the fp32 reference (measured: 1-2 flipped spikes / 67M).
  - The output combine  s1+s2 = (h1>=th1)+(h2>=th2)  is split between:
      * VectorE: fused custom op over the first 640 columns,
      * ScalarE+TensorE over the remaining 1408 columns: Sign(h-th) -> bf16,
        identity-matmul accumulate of the two sign tiles in PSUM (exact on
        {-1,0,1}), then a scaled ACT copy 0.5*psum + 1.0 -> fp8.
    This takes 2/3 of the combine off the Vector engine, which is the
    critical engine (2 STEP passes at 1 elem/lane/cycle are irreducible).
  - Output is written as fp8e4m3 holding s1+s2 in {0,1,2} (exact), 8 MB
    per core instead of 32 MB; the host upcasts and multiplies by 0.5.
    DMA per core: 32 MB in + 8 MB out -> ~114 us at ~350 GB/s; measured
    steady-state kernel time ~135-145 us/core (VectorE/ScalarE-balanced),
    vs ~185 us for the all-VectorE fp32-output version.
"""

import numpy as np
import ml_dtypes  # noqa: F401  (fp8 dtype availability)

import concourse.bacc as bacc
import concourse.mybir as mybir
import concourse.tile as tile
import concourse.dve_ops as dve_ops
from concourse.dve_ops import DveOp
from concourse.dve_spec import (
    C0,
    C1,
    C2,
    Spec,
    Src0,
    Src1,
    Zero,
    _has_src1,
    lower,
    select,
)
from concourse.dve_uop import DveOpSpec
from concourse.masks import make_identity
from concourse import bass_utils

T, B, N = 16, 64, 65536
NCORES = 8
BS = B // NCORES        # batch rows per core
POS = BS * N            # independent positions per core
P = 128
FREE = POS // P         # 4096 columns per partition
CHUNK = 2048
NCHUNK = FREE // CHUNK
ACT_COLS = 1408         # columns of each chunk handled by the ACT+PE path
R = 0.75                # fl32(1 / fl32(4/3)) == 0.75 exactly
TH1, TH2 = 0.75, 1.25
F32 = mybir.dt.float32
BF16 = mybir.dt.bfloat16
FP8 = mybir.dt.float8e4
AF = mybir.ActivationFunctionType


def _register(name: str, spec: Spec) -> DveOp:
    """Register a custom DVE op at runtime (uops sha computed here)."""
    if name in dve_ops._SUB_OPCODE_FOR_NAME:
        for op in dve_ops.OPS:
            if op.name == name:
                return op
    row = dve_ops._CUSTOM_DVE_ROW_BASE + len(dve_ops.OPS)
    assert row < 0x20, "custom DVE opcode rows exhausted"
    sha = {}
    for ver in ("v3", "v4"):
        s = DveOpSpec(name=name, opcode=row, uops=lower(spec, ver=ver),
                      rd1_en=_has_src1(spec))
        sha[ver] = s.sha(ver)
    op = DveOp(name, spec, subdim=False, uops_sha=sha)
    dve_ops.OPS.append(op)
    dve_ops._SUB_OPCODE_FOR_NAME[name] = row
    dve_ops.CUSTOM_DVE_SPECS[name] = spec
    return op


_vp_node = select(Src1 < C1, Src1, Zero)
BILIF_STEP = _register(
    "BILIF_STEP",
    Spec(
        body=(Src0 - _vp_node) * C0 + _vp_node,
        reference=lambda in0, in1, s0, s1, imm2: (
            (in0 - np.where(in1 < s1, in1, 0).astype(np.float32))
            * np.float32(s0)
            + np.where(in1 < s1, in1, 0).astype(np.float32)
        ),
    ),
)
BILIF_OUT = _register(
    "BILIF_OUT",
    Spec(
        body=((Src0 >= C0) + (Src1 >= C1)) * C2,
        reference=lambda in0, in1, s0, s1, imm2: (
            (in0 >= s0).astype(np.float32) + (in1 >= s1).astype(np.float32)
        ) * np.float32(imm2),
    ),
)

_NC_CACHE = {}


def _build_nc(repeat: int = 1, act_cols: int = ACT_COLS):
    """Build + compile the SPMD per-core program. `repeat` replays the body
    (used only for steady-state timing experiments)."""
    key = (repeat, act_cols)
    if key in _NC_CACHE:
        return _NC_CACHE[key]
    dve_cols = CHUNK - act_cols
    nblk = (act_cols + 511) // 512
    nc = bacc.Bacc("TRN2", target_bir_lowering=False, debug=False,
                   num_devices=NCORES)
    x_d = nc.dram_tensor("x", [T * P, FREE], F32, kind="ExternalInput").ap()
    o_d = nc.dram_tensor("o", [T * P, FREE], FP8, kind="ExternalOutput").ap()

    with tile.TileContext(nc) as tc:
        with tc.tile_pool(name="xp", bufs=16) as xp, \
             tc.tile_pool(name="h1p", bufs=3) as h1p, \
             tc.tile_pool(name="h2p", bufs=3) as h2p, \
             tc.tile_pool(name="a1p", bufs=2) as a1p, \
             tc.tile_pool(name="a2p", bufs=2) as a2p, \
             tc.tile_pool(name="outp", bufs=4) as outp, \
             tc.tile_pool(name="psp", bufs=2, space="PSUM") as psp, \
             tc.tile_pool(name="zp", bufs=1) as zp:
            ident = zp.tile([P, P], BF16, tag="ident", name="ident")
            make_identity(nc, ident[:])
            b1 = zp.tile([P, 1], F32, tag="b1", name="b1")
            nc.vector.memset(b1[:], -TH1)
            b2 = zp.tile([P, 1], F32, tag="b2", name="b2")
            nc.vector.memset(b2[:], -TH2)
            for rep in range(repeat):
                for k in range(NCHUNK):
                    c0 = k * CHUNK
                    # Load each x[t] tile once, in first-use order
                    # (fwd uses t at step t, bwd uses t at step 15-t).
                    xt = {}
                    for t in [v for s in range(T // 2) for v in (s, T - 1 - s)]:
                        xt[t] = xp.tile([P, CHUNK], F32, tag="x",
                                        name=f"x{rep}_{k}_{t}")
                        nc.sync.dma_start(
                            out=xt[t][:],
                            in_=x_d[t * P:(t + 1) * P, c0:c0 + CHUNK])
                    h1_prev, h2_prev = None, None
                    for t in range(T):
                        h1 = h1p.tile([P, CHUNK], F32, tag="h1", name="h1")
                        h2 = h2p.tile([P, CHUNK], F32, tag="h2", name="h2")
                        if t == 0:
                            # v = 0: h = 0.75*x exactly; single-src fp32
                            # tensor_scalar streams at 2x (2-port mode)
                            nc.vector.tensor_scalar(
                                out=h1[:], in0=xt[0][:], scalar1=R,
                                scalar2=None, op0=mybir.AluOpType.mult)
                            nc.vector.tensor_scalar(
                                out=h2[:], in0=xt[T - 1][:], scalar1=R,
                                scalar2=None, op0=mybir.AluOpType.mult)
                        else:
                            nc.vector._custom_dve(BILIF_STEP, out=h1[:],
                                                  in0=xt[t][:],
                                                  in1=h1_prev[:],
                                                  s0=R, s1=TH1)
                            nc.vector._custom_dve(BILIF_STEP, out=h2[:],
                                                  in0=xt[T - 1 - t][:],
                                                  in1=h2_prev[:],
                                                  s0=R, s1=TH2)
                        o = outp.tile([P, CHUNK], FP8, tag="o", name="o")
                        if dve_cols:
                            nc.vector._custom_dve(
                                BILIF_OUT, out=o[:, :dve_cols],
                                in0=h1[:, :dve_cols], in1=h2[:, :dve_cols],
                                s0=TH1, s1=TH2, imm2=1.0)
                        if act_cols:
                            a1 = a1p.tile([P, act_cols], BF16, tag="a1",
                                          name="a1")
                            nc.scalar.activation(out=a1[:],
                                                 in_=h1[:, dve_cols:],
                                                 func=AF.Sign, bias=b1[:],
                                                 scale=1.0)
                            a2 = a2p.tile([P, act_cols], BF16, tag="a2",
                                          name="a2")
                            nc.scalar.activation(out=a2[:],
                                                 in_=h2[:, dve_cols:],
                                                 func=AF.Sign, bias=b2[:],
                                                 scale=1.0)
                            ps = psp.tile([P, act_cols], F32, tag="ps",
                                          name="ps")
                            for j in range(nblk):
                                sl = slice(j * 512, min((j + 1) * 512,
                                                        act_cols))
                                nc.tensor.matmul(ps[:, sl], ident[:],
                                                 a1[:, sl], start=True,
                                                 stop=False)
                                nc.tensor.matmul(ps[:, sl], ident[:],
                                                 a2[:, sl], start=False,
                                                 stop=True)
                            # psum = sign(h1-th1)+sign(h2-th2) in {-2..2};
                            # 0.5*psum + 1.0 == s1+s2 except where h == th
                            # exactly (sign(0)=0), ~2 elements per 67M.
                            nc.scalar.activation(out=o[:, dve_cols:],
                                                 in_=ps[:], func=AF.Copy,
                                                 bias=1.0, scale=0.5)
                        nc.sync.dma_start(
                            out=o_d[t * P:(t + 1) * P, c0:c0 + CHUNK],
                            in_=o[:])
                        h1_prev, h2_prev = h1, h2

    nc.compile()
    _NC_CACHE[key] = nc
    return nc


def _run(inputs: np.ndarray, repeat: int = 1, act_cols: int = ACT_COLS,
         **kwargs):
    nc = _build_nc(repeat, act_cols)
    in_maps = []
    for c in range(NCORES):
        shard = np.ascontiguousarray(
            inputs[:, c * BS:(c + 1) * BS, :]).reshape(T * P, FREE)
        in_maps.append({"x": shard})
    return bass_utils.run_bass_kernel_spmd(
        nc, in_maps, core_ids=list(range(NCORES)), **kwargs)


def kernel(inputs: np.ndarray, **kwargs) -> np.ndarray:
    inputs = np.asarray(inputs)
    assert inputs.shape == (T, B, N) and inputs.dtype == np.float32
    res = None
    err = None
    for _attempt in range(3):  # retry transient device faults
        try:
            res = _run(inputs, **kwargs)
            break
        except Exception as e:  # noqa: BLE001
            err = e
    if res is None:
        raise err
    out = np.empty((T, B, N), np.float32)
    for c in range(NCORES):
        out[:, c * BS:(c + 1) * BS, :] = (
            res.results[c]["o"].astype(np.float32) * np.float32(0.5)
        ).reshape(T, BS, N)
    return out

